# revision 24
# baseline (speedup 1.0000x reference)
"""Trainium2 Bass kernel for nn_AdjacencyMatrixLayer.

Computes, for coordinates [B, K, 3] and scalar precision_squared p2:
    d[b,i,j] = ||x_i - x_j||^2
    A = exp(-d * p2)
    out = softmax(A, axis=-1)

Math notes:
  - d_ij = n_i + n_j - 2 x_i.x_j is one PE matmul over 15 bf16-split
    augmented rows (hi/lo splitting keeps d accurate to ~2e-5 while running
    the PE at bf16 rate; plain fp32 matmuls measure ~4x slower).
  - A = exp(-p2*d) in (0, 1], so the softmax needs no max subtraction:
    out_ij = exp(A_ij) / sum_j exp(A_ij).

Pipeline (raw bass, explicit semaphores), strips of 128 rows processed in
pairs so the two exp passes amortize the ~470ns fixed ACTIVATE cost:
  PE : d(pair)   = matmuls -> PSUM[pair%2]           (bf16, 4 matmuls)
  ACT: a(pair)   = exp(-p2*d) in-place in PSUM       (one ACTIVATE per pair)
       e(pair)   = exp(a) -> SBUF                    (one ACTIVATE per pair)
  GP : s(strip)  = rowsum(e) via tensor_scalar accum (idle engine does sums)
  DVE: rec = 1/s ; o(strip) = e * rec
  SP : DMA out strip -> HBM

Sharding: pure data parallel, batch dim B=128 split over 8 cores (16 each).
"""

import os
import sys

import numpy as np

for _p in ("/opt/trn_rl_repo",):
    if _p not in sys.path and os.path.isdir(_p):
        sys.path.insert(0, _p)

import ml_dtypes

import concourse.bass as bass
from concourse import mybir
from concourse.bass_utils import run_bass_kernel_spmd

B, K, D = 128, 1024, 3
N_CORES = 8
B_LOCAL = B // N_CORES  # 16 batches per core
P = 128                 # partition tile
RB = K // P             # 8 row blocks per batch
T = B_LOCAL * RB        # 128 strips per core
NP = T // 2             # 64 strip pairs
AUGR = 15               # bf16-split augmented rows
FP32 = mybir.dt.float32
BF16 = mybir.dt.bfloat16
Exp = mybir.ActivationFunctionType.Exp
MULT = mybir.AluOpType.mult
ADD = mybir.AluOpType.add

N_O = 4      # out buffers

_cached = {}


def _build_bass():
    nc = bass.Bass()

    aug = nc.declare_dram_parameter("aug", [B_LOCAL, AUGR, 2 * K], BF16, isOutput=False)
    consts = nc.declare_dram_parameter("consts", [P, 16], FP32, isOutput=False)
    out = nc.declare_dram_parameter("out", [B_LOCAL, K, K], FP32, isOutput=True)

    with (
        nc.sbuf_tensor([AUGR, B_LOCAL * 2 * K], BF16) as uv,   # all aug data
        nc.sbuf_tensor([P, 16], FP32) as cz,                   # col0=-p2, col1=0
        nc.sbuf_tensor([P, 2 * 2 * K], FP32) as e_buf,         # 2 pair slots
        nc.sbuf_tensor([P, N_O * K], FP32) as o_buf,
        nc.sbuf_tensor([P, 16], FP32) as sr_buf,               # s/rec 4 slots
        nc.psum_tensor([P, 2 * 2 * K], FP32) as d_ps,          # 2 pair slots
        nc.semaphore("sem_in") as sem_in,
        nc.semaphore("sem_dummy") as sem_dummy,
        nc.semaphore("peg_sem") as peg_sem,
        nc.semaphore("act1_sem") as act1_sem,
        nc.semaphore("act2_sem") as act2_sem,
        nc.semaphore("dve_c_sem") as dve_c_sem,
        nc.semaphore("dve1_sem") as dve1_sem,
        nc.semaphore("dve_sem") as dve_sem,
        nc.semaphore("out_sem0") as out_sem0,
        nc.semaphore("out_sem1") as out_sem1,
        nc.semaphore("out_sem2") as out_sem2,
        nc.semaphore("out_sem3") as out_sem3,
        nc.Block() as block,
    ):
        out_sems = [out_sem0, out_sem1, out_sem2, out_sem3]
        aug_r = aug.rearrange("b p c -> p b c")

        def psum_pair(p):
            return d_ps[:, (p % 2) * 2 * K:(p % 2 + 1) * 2 * K]

        def e_pair(p):
            return e_buf[:, (p % 2) * 2 * K:(p % 2 + 1) * 2 * K]

        def e_strip(t):
            pp = (t // 2) % 2
            return e_buf[:, (2 * pp + t % 2) * K:(2 * pp + t % 2 + 1) * K]

        @block.sync
        def _(sync):
            # uv spans 15 partitions -> its DMA may inc the sem by <16.
            # HWDGE drains FIFO per engine, so the 128-partition consts DMA
            # completing implies the earlier uv DMA completed too.
            sync.dma_start(
                uv.rearrange("p (b c) -> p b c", b=B_LOCAL), aug_r
            ).then_inc(sem_dummy, 16)
            sync.dma_start(cz[:], consts[:]).then_inc(sem_in, 16)
            for t in range(T):
                b, r = divmod(t, RB)
                sync.wait_ge(dve_sem, t + 1)
                sync.dma_start(
                    out[b, r * P:(r + 1) * P, :],
                    o_buf[:, (t % N_O) * K:(t % N_O + 1) * K],
                ).then_inc(out_sems[t % N_O], 16)

        @block.tensor
        def _(tensor):
            tensor.wait_ge(sem_in, 16)
            for t in range(T):
                b, r = divmod(t, RB)
                p = t // 2
                if p >= 2 and t % 2 == 0:
                    tensor.wait_ge(act2_sem, p - 1)  # pair psum slot free
                ubase = b * 2 * K
                lhsT = uv[:, ubase + r * P: ubase + (r + 1) * P]
                ps = psum_pair(p)[:, (t % 2) * K:(t % 2 + 1) * K]
                for h in range(2):
                    mm = nc.tensor.matmul(
                        ps[:, h * 512:(h + 1) * 512],
                        lhsT,
                        uv[:, ubase + K + h * 512: ubase + K + (h + 1) * 512],
                        start=True, stop=True,
                    )
                mm.then_inc(peg_sem, 1)

        def _exp2(scalar, p):
            # e = exp(a) for a whole pair
            scalar.wait_ge(act1_sem, p + 1)
            if p >= 2:
                # e slot (p%2) readers from pair p-2: dve copies of its strips
                scalar.wait_ge(dve_c_sem, 2 * (p - 2) + 2)
            nc.scalar.activation(
                e_pair(p), psum_pair(p), Exp, bias=cz[:, 1:2],
            ).then_inc(act2_sem, 1)

        @block.scalar
        def _(scalar):
            scalar.wait_ge(sem_in, 16)
            for p in range(NP):
                scalar.wait_ge(peg_sem, 2 * p + 2)
                ps = psum_pair(p)
                nc.scalar.activation(
                    ps, ps, Exp, bias=cz[:, 1:2], scale=cz[:, 0:1],
                ).then_inc(act1_sem, 1)
                if p >= 1:
                    _exp2(scalar, p - 1)
            _exp2(scalar, NP - 1)

        def _copy_accum(vector, t):
            # o <- e (unnormalized), s <- rowsum(e); both at 2x fp32 SBUF
            vector.wait_ge(act2_sem, t // 2 + 1)
            if t >= N_O:
                # per-slot sem: at most one in-flight DMA each
                vector.wait_ge(out_sems[t % N_O], 16 * ((t - N_O) // N_O + 1))
            if t >= 4:
                vector.wait_ge(dve1_sem, t - 3)  # s slot free (recip t-4)
            s = sr_buf[:, 2 * (t % 4): 2 * (t % 4) + 1]
            o = o_buf[:, (t % N_O) * K:(t % N_O + 1) * K]
            nc.vector.tensor_scalar(
                o, e_strip(t), 1.0, None, MULT, op1=ADD, accum_out=s,
            ).then_inc(dve_c_sem, 1)

        def _recip(vector, t):
            vector.wait_ge(dve_c_sem, t + 1)
            s = sr_buf[:, 2 * (t % 4): 2 * (t % 4) + 1]
            rec = sr_buf[:, 2 * (t % 4) + 1: 2 * (t % 4) + 2]
            nc.vector.reciprocal(rec, s).then_inc(dve1_sem, 1)

        def _norm(vector, t):
            # in-place o *= rec
            vector.wait_ge(dve1_sem, t + 1)
            rec = sr_buf[:, 2 * (t % 4) + 1: 2 * (t % 4) + 2]
            o = o_buf[:, (t % N_O) * K:(t % N_O + 1) * K]
            nc.vector.tensor_scalar_mul(o, o, rec).then_inc(dve_sem, 1)

        @block.vector
        def _(vector):
            # 3-stage interleave: copy_accum(t); recip(t-1); norm(t-2) keeps
            # >=2 ops between same-engine RAW pairs.
            for t in range(T):
                _copy_accum(vector, t)
                if t >= 1:
                    _recip(vector, t - 1)
                if t >= 2:
                    _norm(vector, t - 2)
            _recip(vector, T - 1)
            _norm(vector, T - 2)
            _norm(vector, T - 1)

    return nc


def _prep_inputs(coordinates, precision_squared):
    coords = np.asarray(coordinates, dtype=np.float32)
    p2 = np.float32(np.asarray(precision_squared).reshape(-1)[0])
    bf16 = ml_dtypes.bfloat16

    n = (coords[..., 0] * coords[..., 0]
         + coords[..., 1] * coords[..., 1]
         + coords[..., 2] * coords[..., 2])  # [B, K]

    # hi/lo bf16 splits
    xh = coords.astype(bf16)
    xl = (coords - xh.astype(np.float32)).astype(bf16)
    nh = n.astype(bf16)
    nr = n - nh.astype(np.float32)
    nm = nr.astype(bf16)
    nl = (nr - nm.astype(np.float32)).astype(bf16)

    ones = np.ones_like(n, dtype=bf16)
    zeros = np.zeros_like(n, dtype=bf16)

    def neg2(a):
        return (-2.0 * a.astype(np.float32)).astype(bf16)

    # u rows (contract dim k): d = sum_k u[k,i] * v[k,j].
    # Order keeps partial sums near d (cancellation first): the PE
    # accumulates in row order, so big terms (norm-hi, -2*h.h) lead and
    # small corrections follow, minimizing fp32 accumulation error.
    u_rows = [nh, ones,
              neg2(xh[..., 0]), neg2(xh[..., 1]), neg2(xh[..., 2]),
              nm, ones,
              neg2(xh[..., 0]), neg2(xh[..., 1]), neg2(xh[..., 2]),
              neg2(xl[..., 0]), neg2(xl[..., 1]), neg2(xl[..., 2]),
              nl, ones]
    v_rows = [ones, nh,
              xh[..., 0], xh[..., 1], xh[..., 2],
              ones, nm,
              xl[..., 0], xl[..., 1], xl[..., 2],
              xh[..., 0], xh[..., 1], xh[..., 2],
              ones, nl]
    del zeros
    aug_u = np.stack(u_rows, axis=1)  # [B, 15, K] bf16
    aug_v = np.stack(v_rows, axis=1)
    aug = np.ascontiguousarray(np.concatenate([aug_u, aug_v], axis=2))  # [B,15,2K]

    consts = np.zeros((P, 16), dtype=np.float32)
    consts[:, 0] = -p2

    in_maps = []
    for c in range(N_CORES):
        sl = slice(c * B_LOCAL, (c + 1) * B_LOCAL)
        in_maps.append({
            "aug": np.ascontiguousarray(aug[sl]),
            "consts": consts,
        })
    return in_maps


def _run(inputs, trace=False):
    if "nc" not in _cached:
        _cached["nc"] = _build_bass()
    nc = _cached["nc"]
    in_maps = _prep_inputs(inputs["coordinates"], inputs["precision_squared"])
    res = run_bass_kernel_spmd(nc, in_maps, list(range(N_CORES)), trace=trace)
    outs = [np.asarray(res.results[c]["out"]) for c in range(N_CORES)]
    full = np.concatenate(outs, axis=0)  # [B, K, K]
    return full, res


def kernel(**inputs):
    full, _ = _run(inputs, trace=False)
    return full


# revision 25
# speedup vs baseline: 1.2001x; 1.2001x over previous
"""Trainium2 Bass kernel for nn_AdjacencyMatrixLayer.

Computes, for coordinates [B, K, 3] and scalar precision_squared p2:
    d[b,i,j] = ||x_i - x_j||^2
    A = exp(-d * p2)
    out = softmax(A, axis=-1)

Math notes:
  - d_ij = n_i + n_j - 2 x_i.x_j is one PE matmul over 15 bf16-split
    augmented rows (hi/lo splitting keeps d accurate to ~2e-5 while running
    the PE at bf16 rate; plain fp32 matmuls measure ~4x slower).
  - A = exp(-p2*d) in (0, 1], so the softmax needs no max subtraction:
    out_ij = exp(A_ij) / sum_j exp(A_ij).

Pipeline (raw bass, explicit semaphores), strips of 128 rows processed in
pairs so the two exp passes amortize the ~470ns fixed ACTIVATE cost:
  PE : d(pair)   = matmuls -> PSUM[pair%2]           (bf16, 4 matmuls)
  ACT: a(pair)   = exp(-p2*d) in-place in PSUM       (one ACTIVATE per pair)
       e(pair)   = exp(a) -> SBUF                    (one ACTIVATE per pair)
  GP : s(strip)  = rowsum(e) via tensor_scalar accum (idle engine does sums)
  DVE: rec = 1/s ; o(strip) = e * rec
  SP : DMA out strip -> HBM

Sharding: pure data parallel, batch dim B=128 split over 8 cores (16 each).
"""

import os
import sys

import numpy as np

for _p in ("/opt/trn_rl_repo",):
    if _p not in sys.path and os.path.isdir(_p):
        sys.path.insert(0, _p)

import ml_dtypes

import concourse.bass as bass
from concourse import mybir
from concourse.bass_utils import run_bass_kernel_spmd

B, K, D = 128, 1024, 3
N_CORES = 8
B_LOCAL = B // N_CORES  # 16 batches per core
P = 128                 # partition tile
RB = K // P             # 8 row blocks per batch
T = B_LOCAL * RB        # 128 strips per core
NP = T // 2             # 64 strip pairs
AUGR = 15               # bf16-split augmented rows
FP32 = mybir.dt.float32
BF16 = mybir.dt.bfloat16
Exp = mybir.ActivationFunctionType.Exp
MULT = mybir.AluOpType.mult
ADD = mybir.AluOpType.add

N_O = 4      # out buffers

_cached = {}


def _build_bass():
    nc = bass.Bass()

    aug = nc.declare_dram_parameter("aug", [B_LOCAL, AUGR, 2 * K], BF16, isOutput=False)
    consts = nc.declare_dram_parameter("consts", [P, 16], FP32, isOutput=False)
    out = nc.declare_dram_parameter("out", [B_LOCAL, K, K], FP32, isOutput=True)

    with (
        nc.sbuf_tensor([AUGR, B_LOCAL * 2 * K], BF16) as uv,   # all aug data
        nc.sbuf_tensor([P, 16], FP32) as cz,                   # col0=-p2, col1=0
        nc.sbuf_tensor([P, 2 * RB * K], FP32) as o_buf,        # 2 batch slots
        nc.sbuf_tensor([P, 16], FP32) as sr_buf,               # s/rec 4 slots
        nc.psum_tensor([P, 2 * 2 * K], FP32) as d_ps,          # 2 pair slots
        nc.semaphore("sem_in") as sem_in,
        nc.semaphore("sem_dummy") as sem_dummy,
        nc.semaphore("peg_sem") as peg_sem,
        nc.semaphore("act1_sem") as act1_sem,
        nc.semaphore("act2_sem") as act2_sem,
        nc.semaphore("dve_r_sem") as dve_r_sem,
        nc.semaphore("dve1_sem") as dve1_sem,
        nc.semaphore("dve_sem") as dve_sem,
        nc.semaphore("out_sem0") as out_sem0,
        nc.semaphore("out_sem1") as out_sem1,
        nc.Block() as block,
    ):
        out_sems = [out_sem0, out_sem1]
        aug_r = aug.rearrange("b p c -> p b c")

        def psum_pair(p):
            return d_ps[:, (p % 2) * 2 * K:(p % 2 + 1) * 2 * K]

        def o_pair(p):
            b = p // (RB // 2)
            return o_buf[:, ((b % 2) * RB + (p % (RB // 2)) * 2) * K:
                            ((b % 2) * RB + (p % (RB // 2)) * 2 + 2) * K]

        def o_strip(t):
            b, r = divmod(t, RB)
            return o_buf[:, ((b % 2) * RB + r) * K:((b % 2) * RB + r + 1) * K]

        @block.sync
        def _(sync):
            # uv spans 15 partitions -> its DMA may inc the sem by <16.
            # HWDGE drains FIFO per engine, so the 128-partition consts DMA
            # completing implies the earlier uv DMA completed too.
            sync.dma_start(
                uv.rearrange("p (b c) -> p b c", b=B_LOCAL), aug_r
            ).then_inc(sem_dummy, 16)
            sync.dma_start(cz[:], consts[:]).then_inc(sem_in, 16)
            for b in range(B_LOCAL):
                sync.wait_ge(dve_sem, RB * (b + 1))
                sync.dma_start(
                    out[b].rearrange("(s p) c -> p s c", p=P),
                    o_buf[:, (b % 2) * RB * K:((b % 2) + 1) * RB * K]
                    .rearrange("p (s c) -> p s c", s=RB),
                ).then_inc(out_sems[b % 2], 16)

        @block.tensor
        def _(tensor):
            tensor.wait_ge(sem_in, 16)
            for t in range(T):
                b, r = divmod(t, RB)
                p = t // 2
                if p >= 2 and t % 2 == 0:
                    tensor.wait_ge(act2_sem, p - 1)  # pair psum slot free
                ubase = b * 2 * K
                lhsT = uv[:, ubase + r * P: ubase + (r + 1) * P]
                ps = psum_pair(p)[:, (t % 2) * K:(t % 2 + 1) * K]
                for h in range(2):
                    mm = nc.tensor.matmul(
                        ps[:, h * 512:(h + 1) * 512],
                        lhsT,
                        uv[:, ubase + K + h * 512: ubase + K + (h + 1) * 512],
                        start=True, stop=True,
                    )
                mm.then_inc(peg_sem, 1)

        def _exp2(scalar, p):
            # e = exp(a) for a whole pair, written straight into the batch
            # output buffer (normalized later, in place, by DVE)
            scalar.wait_ge(act1_sem, p + 1)
            b = p // (RB // 2)
            if b >= 2 and p % (RB // 2) == 0:
                # batch slot reused from batch b-2: its DMA must be done
                scalar.wait_ge(out_sems[b % 2], 16 * (b // 2))
            nc.scalar.activation(
                o_pair(p), psum_pair(p), Exp, bias=cz[:, 1:2],
            ).then_inc(act2_sem, 1)

        @block.scalar
        def _(scalar):
            scalar.wait_ge(sem_in, 16)
            for p in range(NP):
                scalar.wait_ge(peg_sem, 2 * p + 2)
                ps = psum_pair(p)
                nc.scalar.activation(
                    ps, ps, Exp, bias=cz[:, 1:2], scale=cz[:, 0:1],
                ).then_inc(act1_sem, 1)
                if p >= 1:
                    _exp2(scalar, p - 1)
            _exp2(scalar, NP - 1)

        def _reduce(vector, t):
            # s <- rowsum(o_strip)   (o holds unnormalized e)
            vector.wait_ge(act2_sem, t // 2 + 1)
            if t >= 4:
                vector.wait_ge(dve1_sem, t - 3)  # s slot free (recip t-4)
            s = sr_buf[:, 2 * (t % 4): 2 * (t % 4) + 1]
            nc.vector.reduce_sum(
                s, o_strip(t), axis=mybir.AxisListType.X,
            ).then_inc(dve_r_sem, 1)

        def _recip(vector, t):
            vector.wait_ge(dve_r_sem, t + 1)
            s = sr_buf[:, 2 * (t % 4): 2 * (t % 4) + 1]
            rec = sr_buf[:, 2 * (t % 4) + 1: 2 * (t % 4) + 2]
            nc.vector.reciprocal(rec, s).then_inc(dve1_sem, 1)

        def _norm(vector, t):
            # in-place o *= rec
            vector.wait_ge(dve1_sem, t + 1)
            rec = sr_buf[:, 2 * (t % 4) + 1: 2 * (t % 4) + 2]
            o = o_strip(t)
            nc.vector.tensor_scalar_mul(o, o, rec).then_inc(dve_sem, 1)

        @block.vector
        def _(vector):
            # 3-stage interleave: reduce(t); recip(t-1); norm(t-2) keeps
            # >=2 ops between same-engine RAW pairs.
            for t in range(T):
                _reduce(vector, t)
                if t >= 1:
                    _recip(vector, t - 1)
                if t >= 2:
                    _norm(vector, t - 2)
            _recip(vector, T - 1)
            _norm(vector, T - 2)
            _norm(vector, T - 1)

    return nc


def _prep_inputs(coordinates, precision_squared):
    coords = np.asarray(coordinates, dtype=np.float32)
    p2 = np.float32(np.asarray(precision_squared).reshape(-1)[0])
    bf16 = ml_dtypes.bfloat16

    n = (coords[..., 0] * coords[..., 0]
         + coords[..., 1] * coords[..., 1]
         + coords[..., 2] * coords[..., 2])  # [B, K]

    # hi/lo bf16 splits
    xh = coords.astype(bf16)
    xl = (coords - xh.astype(np.float32)).astype(bf16)
    nh = n.astype(bf16)
    nr = n - nh.astype(np.float32)
    nm = nr.astype(bf16)
    nl = (nr - nm.astype(np.float32)).astype(bf16)

    ones = np.ones_like(n, dtype=bf16)
    zeros = np.zeros_like(n, dtype=bf16)

    def neg2(a):
        return (-2.0 * a.astype(np.float32)).astype(bf16)

    # u rows (contract dim k): d = sum_k u[k,i] * v[k,j].
    # Order keeps partial sums near d (cancellation first): the PE
    # accumulates in row order, so big terms (norm-hi, -2*h.h) lead and
    # small corrections follow, minimizing fp32 accumulation error.
    u_rows = [nh, ones,
              neg2(xh[..., 0]), neg2(xh[..., 1]), neg2(xh[..., 2]),
              nm, ones,
              neg2(xh[..., 0]), neg2(xh[..., 1]), neg2(xh[..., 2]),
              neg2(xl[..., 0]), neg2(xl[..., 1]), neg2(xl[..., 2]),
              nl, ones]
    v_rows = [ones, nh,
              xh[..., 0], xh[..., 1], xh[..., 2],
              ones, nm,
              xl[..., 0], xl[..., 1], xl[..., 2],
              xh[..., 0], xh[..., 1], xh[..., 2],
              ones, nl]
    del zeros
    aug_u = np.stack(u_rows, axis=1)  # [B, 15, K] bf16
    aug_v = np.stack(v_rows, axis=1)
    aug = np.ascontiguousarray(np.concatenate([aug_u, aug_v], axis=2))  # [B,15,2K]

    consts = np.zeros((P, 16), dtype=np.float32)
    consts[:, 0] = -p2

    in_maps = []
    for c in range(N_CORES):
        sl = slice(c * B_LOCAL, (c + 1) * B_LOCAL)
        in_maps.append({
            "aug": np.ascontiguousarray(aug[sl]),
            "consts": consts,
        })
    return in_maps


def _run(inputs, trace=False):
    if "nc" not in _cached:
        _cached["nc"] = _build_bass()
    nc = _cached["nc"]
    in_maps = _prep_inputs(inputs["coordinates"], inputs["precision_squared"])
    res = run_bass_kernel_spmd(nc, in_maps, list(range(N_CORES)), trace=trace)
    outs = [np.asarray(res.results[c]["out"]) for c in range(N_CORES)]
    full = np.concatenate(outs, axis=0)  # [B, K, K]
    return full, res


def kernel(**inputs):
    full, _ = _run(inputs, trace=False)
    return full
